# revision 24
# baseline (speedup 1.0000x reference)
"""EdgeConv (gather endpoints + concat edge_attr + 2-layer MLP) on 8 trn2 cores.

Edge/data-parallel sharding per the hint: 800k edges split 100k/core (padded
to 102400 = 25 groups x 4096 edges). All MLP compute (bf16 matmuls on PE,
ReLU+bias on ACT, bias add + bf16 cast on DVE) and all bulk data streaming
run on device.

The per-edge endpoint features x[row]/x[col] are prepared by the host as a
feature-major [128, E] bf16 tile stream (rows 0-63 = x[row].T, 64-127 =
x[col].T), exactly like the edge_attr transpose, because this toolchain
cannot bulk-gather on device: the only correctly-lowered indirect-DMA form
is 128 rows/instruction at ~1.5us/instruction (measured on HW in a previous
session), and dma_gather requires int16 indices (node ids reach 50000).

All streams are bf16 (tolerance is 2e-2; bf16 end-to-end measures 5.3e-3,
fp8 variants measure 1.9-3.0e-2 and are rejected), halving HBM traffic vs
fp32. Every DMA moves a full 128-partition tile so all 16 SDMA engines
engage. Shards are 24 groups of 4096 edges plus one trailing group of 2048
(pad 100000 -> 100352, 0.35%):
  xg  [G, 128, 4096]  gathered endpoint features, feature-major
  ea2 [G, 128, 2048]  edge_attr.T with the group's two half-group edge
                      runs stacked on the partition axis
  out [G, 128, 2048]  output, feature-major, same half-stacking as ea2
  (+ xgl/eal/outl half-size tensors for the trailing group)

Per superblock pair p (even = edges [512p, 512p+512) of the group's first
half, odd = same slice of the second half), every matmul runs N=512 with
all 128 PE rows+columns live:
  ps1[0:64]   = W1[0:128].T @ xg_even     (K=128, PE tile (0,0))
  ps1[64:128] = W1[0:128].T @ xg_odd      (K=128, PE tile (0,64))
  ps1[:]     += blkdiag(W1c,W1c).T @ ea[:, 512-col slice]
                (one K=128 matmul covers BOTH halves' edge_attr term)
  h1[128,512] = relu(ps1 + b1)            (one ACT op per 1024 edges)
  ps2[:]      = blkdiag(W2,W2).T @ h1     (one K=128 matmul, both halves)
  out_t[:, sl] = ps2 + b2                 (DVE per-partition scalar add,
                                           f32 psum -> bf16 sbuf)
Layer 2 of each superblock is emitted AFTER the next superblock's layer-1
matmuls (software pipelining): the PE's in-order queue then never
head-of-line blocks waiting for the ACT relu, and each group's output
store is emitted with its last superblock's deferred flush. Measured
engine budget per pass: DMA 174us (the bound), PE ~85us, ACT/DVE ~55us
each; the kernel times at the measured DMA-only floor.

The host inverts the layout (transpose + unpad + f32 upcast) when
assembling the full [800000, 64] result. DMA split: xg + ea2 loads on the
sync HWDGE ring, out stores on the scalar HWDGE ring.
"""

import sys

sys.path.insert(0, "/opt/trn_rl_repo")

import contextlib

import numpy as np
from ml_dtypes import bfloat16

import concourse.bass as bass
import concourse.bacc as bacc
import concourse.mybir as mybir
import concourse.tile as tile
from concourse import bass_utils

N_NODES = 50000
N_EDGES = 800000
D = 64
P = 128
N_CORES = 8
E_SHARD = N_EDGES // N_CORES          # 100000
GROUP = 4096                          # edges per full group
G = E_SHARD // GROUP                  # 24 full groups
GROUP_L = 2048                        # trailing group (pad 100000 -> 100352)
HALF = GROUP // 2                     # 2048
HALF_L = GROUP_L // 2                 # 1024
E_PAD = G * GROUP + GROUP_L           # 100352
SBW = 512                             # edges per superblock

F32 = mybir.dt.float32
BF16 = mybir.dt.bfloat16


def build_program(n_groups=G, n_reps=1):
    nc = bacc.Bacc(
        "TRN2",
        target_bir_lowering=False,
        debug=False,
        enable_asserts=False,
        num_devices=N_CORES,
    )
    t_xg = nc.dram_tensor(
        "xg", [n_groups, P, GROUP], BF16, kind="ExternalInput"
    ).ap()
    t_xgl = nc.dram_tensor("xgl", [P, GROUP_L], BF16, kind="ExternalInput").ap()
    t_ea2 = nc.dram_tensor(
        "ea2", [n_groups, P, HALF], BF16, kind="ExternalInput"
    ).ap()
    t_eal = nc.dram_tensor("eal", [P, HALF_L], BF16, kind="ExternalInput").ap()
    t_w1ab = nc.dram_tensor("w1ab", [P, D], BF16, kind="ExternalInput").ap()
    t_w1c2 = nc.dram_tensor("w1c2", [P, P], BF16, kind="ExternalInput").ap()
    t_w22 = nc.dram_tensor("w22", [P, P], BF16, kind="ExternalInput").ap()
    t_b1d = nc.dram_tensor("b1d", [P, 1], F32, kind="ExternalInput").ap()
    t_b2d = nc.dram_tensor("b2d", [P, 1], F32, kind="ExternalInput").ap()
    t_out = nc.dram_tensor(
        "out", [n_groups, P, HALF], BF16, kind="ExternalOutput"
    ).ap()
    t_outl = nc.dram_tensor("outl", [P, HALF_L], BF16, kind="ExternalOutput").ap()

    with tile.TileContext(nc) as tc:
        with (
            tc.tile_pool(name="consts", bufs=1) as consts,
            tc.tile_pool(name="gxp", bufs=4) as gxp,
            tc.tile_pool(name="eap", bufs=4) as eap,
            tc.tile_pool(name="h1p", bufs=4) as h1p,
            tc.tile_pool(name="outp", bufs=4) as outp,
            tc.tile_pool(name="ps1", bufs=4, space="PSUM") as ps1p,
            tc.tile_pool(name="ps2", bufs=4, space="PSUM") as ps2p,
        ):
            w1ab = consts.tile_from(t_w1ab)
            w1c2 = consts.tile_from(t_w1c2)
            w22 = consts.tile_from(t_w22)
            b1d = consts.tile_from(t_b1d)
            b2d = consts.tile_from(t_b2d)

            def l2_flush(h1, out_t, sl, store):
                """Deferred layer-2 for one superblock: by emission time the
                ReLU producing h1 has already overlapped with the next
                superblock's L1 matmuls, so the PE never head-of-line
                stalls on the ACT engine. The group's output store rides
                with its last superblock's flush (Tile orders by emission,
                so the store must be emitted after the final DVE write)."""
                ps2 = ps2p.tile([P, SBW], F32, tag="p2")
                nc.tensor.matmul(
                    ps2[:], lhsT=w22[:], rhs=h1[:],
                    start=True, stop=True,
                )
                nc.vector.tensor_scalar_add(
                    out=out_t[:, sl], in0=ps2[:], scalar1=b2d[:]
                )
                if store is not None:
                    nc.scalar.dma_start(out=store, in_=out_t[:])

            rep_ctx = (
                tc.For_i(0, n_reps, 1) if n_reps > 1 else contextlib.nullcontext()
            )
            with rep_ctx:
                pend = None
                groups = [
                    (t_xg[g], t_ea2[g], t_out[g], GROUP, HALF, "")
                    for g in range(n_groups)
                ] + [(t_xgl, t_eal, t_outl, GROUP_L, HALF_L, "l")]
                for xg_src, ea_src, out_dst, grp, half, sfx in groups:
                    xg = gxp.tile([P, grp], BF16, tag="gx" + sfx)
                    nc.sync.dma_start(out=xg[:], in_=xg_src)
                    ea = eap.tile([P, half], BF16, tag="ea" + sfx)
                    nc.scalar.dma_start(out=ea[:], in_=ea_src)
                    out_t = outp.tile([P, half], BF16, tag="out" + sfx)
                    for p in range(half // SBW):
                        sl = slice(SBW * p, SBW * (p + 1))
                        ps1 = ps1p.tile([P, SBW], F32, tag="p1")
                        nc.tensor.matmul(
                            ps1[0:D], lhsT=w1ab[:], rhs=xg[:, sl],
                            start=True, stop=False,
                        )
                        sl_o = slice(half + SBW * p, half + SBW * (p + 1))
                        nc.tensor.matmul(
                            ps1[D:P], lhsT=w1ab[:], rhs=xg[:, sl_o],
                            start=True, stop=False, skip_group_check=True,
                        )
                        nc.tensor.matmul(
                            ps1[:], lhsT=w1c2[:], rhs=ea[:, sl],
                            start=False, stop=True, skip_group_check=True,
                        )
                        h1 = h1p.tile([P, SBW], BF16, tag="h1")
                        nc.scalar.activation(
                            h1[:], ps1[:], mybir.ActivationFunctionType.Relu,
                            bias=b1d[:], scale=1.0,
                        )
                        if pend is not None:
                            l2_flush(*pend)
                        store = out_dst if p == half // SBW - 1 else None
                        pend = (h1, out_t, sl, store)
                if pend is not None:
                    l2_flush(*pend)
                    pend = None

    nc.compile()
    return nc


def make_in_maps(x, edge_attr, W1, b1, W2, b2, edge_index, n_groups=G,
                 e_shard=E_SHARD):
    """Host-side shard/layout prep. Returns per-core input dicts."""
    e_pad = n_groups * GROUP + GROUP_L
    row = np.asarray(edge_index[0], dtype=np.int64)
    col = np.asarray(edge_index[1], dtype=np.int64)
    x16 = np.asarray(x, dtype=np.float32).astype(bfloat16)
    ea16 = np.asarray(edge_attr, dtype=np.float32).astype(bfloat16)
    W1 = np.asarray(W1, dtype=np.float32)
    w1ab = np.ascontiguousarray(W1[:P].astype(bfloat16))

    def blockdiag(w):
        bd = np.zeros((P, P), bfloat16)
        bd[:D, :D] = w
        bd[D:, D:] = w
        return bd

    w1c2 = blockdiag(W1[P:].astype(bfloat16))
    w22 = blockdiag(np.asarray(W2, dtype=np.float32).astype(bfloat16))
    b1d = np.ascontiguousarray(
        np.tile(np.asarray(b1, dtype=np.float32).reshape(D, 1), (2, 1))
    )
    b2d = np.ascontiguousarray(
        np.tile(np.asarray(b2, dtype=np.float32).reshape(D, 1), (2, 1))
    )
    xT16 = np.ascontiguousarray(x16.T)  # [64, N] for fast column gathers

    def half_stack(ea_s, ngr, half):
        """[E', D] -> [ngr, 128, half]: per group, the two half-group edge
        runs stacked on the partition axis, feature-major."""
        return np.ascontiguousarray(
            ea_s.T.reshape(D, ngr, 2, half)
            .transpose(1, 2, 0, 3)
            .reshape(ngr, P, half)
        )

    e_full = n_groups * GROUP
    in_maps = []
    for c in range(N_CORES):
        sl = slice(c * e_shard, (c + 1) * e_shard)
        row_s = np.zeros(e_pad, np.int64)
        row_s[:e_shard] = row[sl]
        col_s = np.zeros(e_pad, np.int64)
        col_s[:e_shard] = col[sl]
        ea_s = np.zeros((e_pad, D), bfloat16)
        ea_s[:e_shard] = ea16[sl]
        ea2 = half_stack(ea_s[:e_full], n_groups, HALF)
        eal = half_stack(ea_s[e_full:], 1, HALF_L)[0]
        # [G, 128, GROUP]: rows 0-63 = x[row].T, rows 64-127 = x[col].T.
        xg = np.empty((n_groups, P, GROUP), bfloat16)
        rs = row_s[:e_full].reshape(n_groups, GROUP)
        cs = col_s[:e_full].reshape(n_groups, GROUP)
        for g in range(n_groups):
            xg[g, :D] = xT16[:, rs[g]]
            xg[g, D:] = xT16[:, cs[g]]
        xgl = np.empty((P, GROUP_L), bfloat16)
        xgl[:D] = xT16[:, row_s[e_full:]]
        xgl[D:] = xT16[:, col_s[e_full:]]
        in_maps.append({
            "xg": xg,
            "xgl": xgl,
            "ea2": ea2,
            "eal": eal,
            "w1ab": w1ab,
            "w1c2": w1c2,
            "w22": w22,
            "b1d": b1d,
            "b2d": b2d,
        })
    return in_maps


def assemble_output(results, n_groups=G, e_shard=E_SHARD):
    """Invert the feature-major half-stacked layout, concatenate shards."""

    def unstack(o, ngr, half):
        return (
            o.reshape(ngr, 2, D, half // SBW, SBW)
            .transpose(0, 1, 3, 4, 2)
            .reshape(ngr * 2 * half, D)
        )

    outs = []
    for c in range(N_CORES):
        o = unstack(results[c]["out"], n_groups, HALF)
        ol = unstack(results[c]["outl"][None], 1, HALF_L)
        outs.append(np.concatenate([o, ol], axis=0)[:e_shard].astype(np.float32))
    return np.ascontiguousarray(np.concatenate(outs, axis=0))


_NC = None
last_results = None


def kernel(x, edge_attr, W1, b1, W2, b2, edge_index, edge_type):
    global _NC, last_results
    if _NC is None:
        _NC = build_program()
    in_maps = make_in_maps(x, edge_attr, W1, b1, W2, b2, edge_index)
    res = bass_utils.run_bass_kernel_spmd(
        _NC, in_maps, core_ids=list(range(N_CORES))
    )
    last_results = res
    return assemble_output(res.results)


# revision 25
# speedup vs baseline: 1.1878x; 1.1878x over previous
"""EdgeConv (gather endpoints + concat edge_attr + 2-layer MLP) on 8 trn2 cores.

Edge/data-parallel sharding per the hint: 800k edges split 100k/core (padded
to 102400 = 25 groups x 4096 edges). All MLP compute (bf16 matmuls on PE,
ReLU+bias on ACT, bias add + bf16 cast on DVE) and all bulk data streaming
run on device.

The per-edge endpoint features x[row]/x[col] are prepared by the host as a
feature-major [128, E] bf16 tile stream (rows 0-63 = x[row].T, 64-127 =
x[col].T), exactly like the edge_attr transpose, because this toolchain
cannot bulk-gather on device: the only correctly-lowered indirect-DMA form
is 128 rows/instruction at ~1.5us/instruction (measured on HW in a previous
session), and dma_gather requires int16 indices (node ids reach 50000).

All streams are bf16 (tolerance is 2e-2; bf16 end-to-end measures 5.3e-3,
fp8 variants measure 1.9-3.0e-2 and are rejected), halving HBM traffic vs
fp32. Every DMA moves a full 128-partition tile so all 16 SDMA engines
engage. Shards are 24 groups of 4096 edges plus one trailing group of 2048
(pad 100000 -> 100352, 0.35%):
  xg  [G, 128, 4096]  gathered endpoint features, feature-major
  ea2 [G, 128, 2048]  edge_attr.T with the group's two half-group edge
                      runs stacked on the partition axis
  out [G, 128, 2048]  output, feature-major, same half-stacking as ea2
  (+ xgl/eal/outl half-size tensors for the trailing group)

Per superblock pair p (even = edges [512p, 512p+512) of the group's first
half, odd = same slice of the second half), every matmul runs N=512 with
all 128 PE rows+columns live:
  ps1[0:64]   = W1[0:128].T @ xg_even     (K=128, PE tile (0,0))
  ps1[64:128] = W1[0:128].T @ xg_odd      (K=128, PE tile (0,64))
  ps1[:]     += blkdiag(W1c,W1c).T @ ea[:, 512-col slice]
                (one K=128 matmul covers BOTH halves' edge_attr term)
  h1[128,512] = relu(ps1 + b1)            (one ACT op per 1024 edges)
  ps2[:]      = blkdiag(W2,W2).T @ h1     (one K=128 matmul, both halves)
  out_t[:, sl] = ps2 + b2                 (DVE per-partition scalar add,
                                           f32 psum -> bf16 sbuf)
Layer 2 of each superblock is emitted AFTER the next superblock's layer-1
matmuls (software pipelining): the PE's in-order queue then never
head-of-line blocks waiting for the ACT relu, and each group's output
store is emitted with its last superblock's deferred flush. Measured
engine budget per pass: DMA 174us (the bound), PE ~85us, ACT/DVE ~55us
each; the kernel times at the measured DMA-only floor.

The host inverts the layout (transpose + unpad + f32 upcast) when
assembling the full [800000, 64] result. DMA split: xg + ea2 loads on the
sync HWDGE ring, out stores on the scalar HWDGE ring.
"""

import sys

sys.path.insert(0, "/opt/trn_rl_repo")

import contextlib

import numpy as np
from ml_dtypes import bfloat16

import concourse.bass as bass
import concourse.bacc as bacc
import concourse.mybir as mybir
import concourse.tile as tile
from concourse import bass_utils

N_NODES = 50000
N_EDGES = 800000
D = 64
P = 128
N_CORES = 8
E_SHARD = N_EDGES // N_CORES          # 100000
GROUP = 4096                          # edges per full group
G = E_SHARD // GROUP                  # 24 full groups
GROUP_L = 2048                        # trailing group (pad 100000 -> 100352)
HALF = GROUP // 2                     # 2048
HALF_L = GROUP_L // 2                 # 1024
E_PAD = G * GROUP + GROUP_L           # 100352
SBW = 512                             # edges per superblock

F32 = mybir.dt.float32
BF16 = mybir.dt.bfloat16


def build_program(n_groups=G, n_reps=1):
    nc = bacc.Bacc(
        "TRN2",
        target_bir_lowering=False,
        debug=False,
        enable_asserts=False,
        num_devices=N_CORES,
    )
    t_xg = nc.dram_tensor(
        "xg", [n_groups, P, GROUP], BF16, kind="ExternalInput"
    ).ap()
    t_xgl = nc.dram_tensor("xgl", [P, GROUP_L], BF16, kind="ExternalInput").ap()
    t_ea2 = nc.dram_tensor(
        "ea2", [n_groups, P, HALF], BF16, kind="ExternalInput"
    ).ap()
    t_eal = nc.dram_tensor("eal", [P, HALF_L], BF16, kind="ExternalInput").ap()
    t_w1ab = nc.dram_tensor("w1ab", [P, D], BF16, kind="ExternalInput").ap()
    t_w1c2 = nc.dram_tensor("w1c2", [P, P], BF16, kind="ExternalInput").ap()
    t_w22 = nc.dram_tensor("w22", [P, P], BF16, kind="ExternalInput").ap()
    t_b1d = nc.dram_tensor("b1d", [P, 1], F32, kind="ExternalInput").ap()
    t_b2d = nc.dram_tensor("b2d", [P, 1], F32, kind="ExternalInput").ap()
    t_out = nc.dram_tensor(
        "out", [n_groups, P, HALF], BF16, kind="ExternalOutput"
    ).ap()
    t_outl = nc.dram_tensor("outl", [P, HALF_L], BF16, kind="ExternalOutput").ap()

    with tile.TileContext(nc) as tc:
        with (
            tc.tile_pool(name="consts", bufs=1) as consts,
            tc.tile_pool(name="gxp", bufs=4) as gxp,
            tc.tile_pool(name="eap", bufs=4) as eap,
            tc.tile_pool(name="h1p", bufs=4) as h1p,
            tc.tile_pool(name="outp", bufs=4) as outp,
            tc.tile_pool(name="ps1", bufs=4, space="PSUM") as ps1p,
            tc.tile_pool(name="ps2", bufs=4, space="PSUM") as ps2p,
        ):
            w1ab = consts.tile_from(t_w1ab)
            w1c2 = consts.tile_from(t_w1c2)
            w22 = consts.tile_from(t_w22)
            b1d = consts.tile_from(t_b1d)
            b2d = consts.tile_from(t_b2d)

            def l2_flush(h1, out_t, sl, store):
                """Deferred layer-2 for one superblock: by emission time the
                ReLU producing h1 has already overlapped with the next
                superblock's L1 matmuls, so the PE never head-of-line
                stalls on the ACT engine. The group's output store rides
                with its last superblock's flush (Tile orders by emission,
                so the store must be emitted after the final DVE write)."""
                ps2 = ps2p.tile([P, SBW], F32, tag="p2")
                nc.tensor.matmul(
                    ps2[:], lhsT=w22[:], rhs=h1[:],
                    start=True, stop=True,
                )
                nc.vector.tensor_scalar_add(
                    out=out_t[:, sl], in0=ps2[:], scalar1=b2d[:]
                )
                if store is not None:
                    nc.scalar.dma_start(out=store, in_=out_t[:])

            rep_ctx = (
                tc.For_i(0, n_reps, 1) if n_reps > 1 else contextlib.nullcontext()
            )
            with rep_ctx:
                pend = None
                groups = [
                    (t_xg[g], t_ea2[g], t_out[g], GROUP, HALF, "")
                    for g in range(n_groups)
                ] + [(t_xgl, t_eal, t_outl, GROUP_L, HALF_L, "l")]
                for xg_src, ea_src, out_dst, grp, half, sfx in groups:
                    xg = gxp.tile([P, grp], BF16, tag="gx" + sfx)
                    nc.sync.dma_start(out=xg[:], in_=xg_src)
                    ea = eap.tile([P, half], BF16, tag="ea" + sfx)
                    nc.sync.dma_start(out=ea[:], in_=ea_src)
                    out_t = outp.tile([P, half], BF16, tag="out" + sfx)
                    for p in range(half // SBW):
                        sl = slice(SBW * p, SBW * (p + 1))
                        ps1 = ps1p.tile([P, SBW], F32, tag="p1")
                        nc.tensor.matmul(
                            ps1[0:D], lhsT=w1ab[:], rhs=xg[:, sl],
                            start=True, stop=False,
                        )
                        sl_o = slice(half + SBW * p, half + SBW * (p + 1))
                        nc.tensor.matmul(
                            ps1[D:P], lhsT=w1ab[:], rhs=xg[:, sl_o],
                            start=True, stop=False, skip_group_check=True,
                        )
                        nc.tensor.matmul(
                            ps1[:], lhsT=w1c2[:], rhs=ea[:, sl],
                            start=False, stop=True, skip_group_check=True,
                        )
                        h1 = h1p.tile([P, SBW], BF16, tag="h1")
                        nc.scalar.activation(
                            h1[:], ps1[:], mybir.ActivationFunctionType.Relu,
                            bias=b1d[:], scale=1.0,
                        )
                        if pend is not None:
                            l2_flush(*pend)
                        store = out_dst if p == half // SBW - 1 else None
                        pend = (h1, out_t, sl, store)
                if pend is not None:
                    l2_flush(*pend)
                    pend = None

    nc.compile()
    return nc


def make_in_maps(x, edge_attr, W1, b1, W2, b2, edge_index, n_groups=G,
                 e_shard=E_SHARD):
    """Host-side shard/layout prep. Returns per-core input dicts."""
    e_pad = n_groups * GROUP + GROUP_L
    row = np.asarray(edge_index[0], dtype=np.int64)
    col = np.asarray(edge_index[1], dtype=np.int64)
    x16 = np.asarray(x, dtype=np.float32).astype(bfloat16)
    ea16 = np.asarray(edge_attr, dtype=np.float32).astype(bfloat16)
    W1 = np.asarray(W1, dtype=np.float32)
    w1ab = np.ascontiguousarray(W1[:P].astype(bfloat16))

    def blockdiag(w):
        bd = np.zeros((P, P), bfloat16)
        bd[:D, :D] = w
        bd[D:, D:] = w
        return bd

    w1c2 = blockdiag(W1[P:].astype(bfloat16))
    w22 = blockdiag(np.asarray(W2, dtype=np.float32).astype(bfloat16))
    b1d = np.ascontiguousarray(
        np.tile(np.asarray(b1, dtype=np.float32).reshape(D, 1), (2, 1))
    )
    b2d = np.ascontiguousarray(
        np.tile(np.asarray(b2, dtype=np.float32).reshape(D, 1), (2, 1))
    )
    xT16 = np.ascontiguousarray(x16.T)  # [64, N] for fast column gathers

    def half_stack(ea_s, ngr, half):
        """[E', D] -> [ngr, 128, half]: per group, the two half-group edge
        runs stacked on the partition axis, feature-major."""
        return np.ascontiguousarray(
            ea_s.T.reshape(D, ngr, 2, half)
            .transpose(1, 2, 0, 3)
            .reshape(ngr, P, half)
        )

    e_full = n_groups * GROUP
    in_maps = []
    for c in range(N_CORES):
        sl = slice(c * e_shard, (c + 1) * e_shard)
        row_s = np.zeros(e_pad, np.int64)
        row_s[:e_shard] = row[sl]
        col_s = np.zeros(e_pad, np.int64)
        col_s[:e_shard] = col[sl]
        ea_s = np.zeros((e_pad, D), bfloat16)
        ea_s[:e_shard] = ea16[sl]
        ea2 = half_stack(ea_s[:e_full], n_groups, HALF)
        eal = half_stack(ea_s[e_full:], 1, HALF_L)[0]
        # [G, 128, GROUP]: rows 0-63 = x[row].T, rows 64-127 = x[col].T.
        xg = np.empty((n_groups, P, GROUP), bfloat16)
        rs = row_s[:e_full].reshape(n_groups, GROUP)
        cs = col_s[:e_full].reshape(n_groups, GROUP)
        for g in range(n_groups):
            xg[g, :D] = xT16[:, rs[g]]
            xg[g, D:] = xT16[:, cs[g]]
        xgl = np.empty((P, GROUP_L), bfloat16)
        xgl[:D] = xT16[:, row_s[e_full:]]
        xgl[D:] = xT16[:, col_s[e_full:]]
        in_maps.append({
            "xg": xg,
            "xgl": xgl,
            "ea2": ea2,
            "eal": eal,
            "w1ab": w1ab,
            "w1c2": w1c2,
            "w22": w22,
            "b1d": b1d,
            "b2d": b2d,
        })
    return in_maps


def assemble_output(results, n_groups=G, e_shard=E_SHARD):
    """Invert the feature-major half-stacked layout, concatenate shards."""

    def unstack(o, ngr, half):
        return (
            o.reshape(ngr, 2, D, half // SBW, SBW)
            .transpose(0, 1, 3, 4, 2)
            .reshape(ngr * 2 * half, D)
        )

    outs = []
    for c in range(N_CORES):
        o = unstack(results[c]["out"], n_groups, HALF)
        ol = unstack(results[c]["outl"][None], 1, HALF_L)
        outs.append(np.concatenate([o, ol], axis=0)[:e_shard].astype(np.float32))
    return np.ascontiguousarray(np.concatenate(outs, axis=0))


_NC = None
last_results = None


def kernel(x, edge_attr, W1, b1, W2, b2, edge_index, edge_type):
    global _NC, last_results
    if _NC is None:
        _NC = build_program()
    in_maps = make_in_maps(x, edge_attr, W1, b1, W2, b2, edge_index)
    res = bass_utils.run_bass_kernel_spmd(
        _NC, in_maps, core_ids=list(range(N_CORES))
    )
    last_results = res
    return assemble_output(res.results)


# revision 26
# speedup vs baseline: 1.2410x; 1.0448x over previous
"""EdgeConv (gather endpoints + concat edge_attr + 2-layer MLP) on 8 trn2 cores.

Edge/data-parallel sharding per the hint: 800k edges split 100k/core (padded
to 102400 = 25 groups x 4096 edges). All MLP compute (bf16 matmuls on PE,
ReLU+bias on ACT, bias add + bf16 cast on DVE) and all bulk data streaming
run on device.

The per-edge endpoint features x[row]/x[col] are prepared by the host as a
feature-major [128, E] bf16 tile stream (rows 0-63 = x[row].T, 64-127 =
x[col].T), exactly like the edge_attr transpose, because this toolchain
cannot bulk-gather on device: the only correctly-lowered indirect-DMA form
is 128 rows/instruction at ~1.5us/instruction (measured on HW in a previous
session), and dma_gather requires int16 indices (node ids reach 50000).

All streams are bf16 (tolerance is 2e-2; bf16 end-to-end measures 5.3e-3,
fp8 variants measure 1.9-3.0e-2 and are rejected), halving HBM traffic vs
fp32. Every DMA moves a full 128-partition tile so all 16 SDMA engines
engage. Shards are 24 groups of 4096 edges plus one trailing group of 2048
(pad 100000 -> 100352, 0.35%):
  xg  [G, 128, 4096]  gathered endpoint features, feature-major
  ea2 [G, 128, 2048]  edge_attr.T with the group's two half-group edge
                      runs stacked on the partition axis
  out [G, 128, 2048]  output, feature-major, same half-stacking as ea2
  (+ xgl/eal/outl half-size tensors for the trailing group)

Per superblock pair p (even = edges [512p, 512p+512) of the group's first
half, odd = same slice of the second half), every matmul runs N=512 with
all 128 PE rows+columns live:
  ps1[0:64]   = W1[0:128].T @ xg_even     (K=128, PE tile (0,0))
  ps1[64:128] = W1[0:128].T @ xg_odd      (K=128, PE tile (0,64))
  ps1[:]     += blkdiag(W1c,W1c).T @ ea[:, 512-col slice]
                (one K=128 matmul covers BOTH halves' edge_attr term)
  h1[128,512] = relu(ps1 + b1)            (one ACT op per 1024 edges)
  ps2[:]      = blkdiag(W2,W2).T @ h1     (one K=128 matmul, both halves)
  out_t[:, sl] = ps2 + b2                 (DVE per-partition scalar add,
                                           f32 psum -> bf16 sbuf)
Layer 2 of each superblock is emitted AFTER the next superblock's layer-1
matmuls (software pipelining): the PE's in-order queue then never
head-of-line blocks waiting for the ACT relu, and each group's output
store is emitted with its last superblock's deferred flush. Measured
engine budget per pass: DMA 174us (the bound), PE ~85us, ACT/DVE ~55us
each; the kernel times at the measured DMA-only floor.

The host inverts the layout (transpose + unpad + f32 upcast) when
assembling the full [800000, 64] result. DMA split: xg + ea2 loads on the
sync HWDGE ring, out stores on the scalar HWDGE ring.
"""

import sys

sys.path.insert(0, "/opt/trn_rl_repo")

import contextlib

import numpy as np
from ml_dtypes import bfloat16

import concourse.bass as bass
import concourse.bacc as bacc
import concourse.mybir as mybir
import concourse.tile as tile
from concourse import bass_utils

N_NODES = 50000
N_EDGES = 800000
D = 64
P = 128
N_CORES = 8
E_SHARD = N_EDGES // N_CORES          # 100000
GROUP = 4096                          # edges per full group
G = E_SHARD // GROUP                  # 24 full groups
GROUP_L = 2048                        # trailing group (pad 100000 -> 100352)
HALF = GROUP // 2                     # 2048
HALF_L = GROUP_L // 2                 # 1024
E_PAD = G * GROUP + GROUP_L           # 100352
SBW = 512                             # edges per superblock

F32 = mybir.dt.float32
BF16 = mybir.dt.bfloat16


def build_program(n_groups=G, n_reps=1):
    nc = bacc.Bacc(
        "TRN2",
        target_bir_lowering=False,
        debug=False,
        enable_asserts=False,
        num_devices=N_CORES,
    )
    t_xg = nc.dram_tensor(
        "xg", [n_groups, P, GROUP], BF16, kind="ExternalInput"
    ).ap()
    t_xgl = nc.dram_tensor("xgl", [P, GROUP_L], BF16, kind="ExternalInput").ap()
    t_ea2 = nc.dram_tensor(
        "ea2", [n_groups, P, HALF], BF16, kind="ExternalInput"
    ).ap()
    t_eal = nc.dram_tensor("eal", [P, HALF_L], BF16, kind="ExternalInput").ap()
    t_w1ab = nc.dram_tensor("w1ab", [P, D], BF16, kind="ExternalInput").ap()
    t_w1c2 = nc.dram_tensor("w1c2", [P, P], BF16, kind="ExternalInput").ap()
    t_w22 = nc.dram_tensor("w22", [P, P], BF16, kind="ExternalInput").ap()
    t_b1d = nc.dram_tensor("b1d", [P, 1], F32, kind="ExternalInput").ap()
    t_b2d = nc.dram_tensor("b2d", [P, 1], F32, kind="ExternalInput").ap()
    t_out = nc.dram_tensor(
        "out", [n_groups, P, HALF], BF16, kind="ExternalOutput"
    ).ap()
    t_outl = nc.dram_tensor("outl", [P, HALF_L], BF16, kind="ExternalOutput").ap()

    with tile.TileContext(nc) as tc:
        with (
            tc.tile_pool(name="consts", bufs=1) as consts,
            tc.tile_pool(name="gxp", bufs=3) as gxp,
            tc.tile_pool(name="eap", bufs=3) as eap,
            tc.tile_pool(name="h1p", bufs=4) as h1p,
            tc.tile_pool(name="outp", bufs=3) as outp,
            tc.tile_pool(name="ps1", bufs=3, space="PSUM") as ps1p,
            tc.tile_pool(name="ps2", bufs=3, space="PSUM") as ps2p,
        ):
            w1ab = consts.tile_from(t_w1ab)
            w1c2 = consts.tile_from(t_w1c2)
            w22 = consts.tile_from(t_w22)
            b1d = consts.tile_from(t_b1d)
            b2d = consts.tile_from(t_b2d)

            def l2_flush(h1, out_t, sl, store):
                """Deferred layer-2 for one superblock: by emission time the
                ReLU producing h1 has already overlapped with the next
                superblock's L1 matmuls, so the PE never head-of-line
                stalls on the ACT engine. The group's output store rides
                with its last superblock's flush (Tile orders by emission,
                so the store must be emitted after the final DVE write)."""
                ps2 = ps2p.tile([P, SBW], F32, tag="p2")
                nc.tensor.matmul(
                    ps2[:], lhsT=w22[:], rhs=h1[:],
                    start=True, stop=True,
                )
                nc.vector.tensor_scalar_add(
                    out=out_t[:, sl], in0=ps2[:], scalar1=b2d[:]
                )
                if store is not None:
                    nc.scalar.dma_start(out=store, in_=out_t[:])

            rep_ctx = (
                tc.For_i(0, n_reps, 1) if n_reps > 1 else contextlib.nullcontext()
            )
            with rep_ctx:
                pend = None
                groups = [
                    (t_xg[g], t_ea2[g], t_out[g], GROUP, HALF, "")
                    for g in range(n_groups)
                ] + [(t_xgl, t_eal, t_outl, GROUP_L, HALF_L, "l")]
                for xg_src, ea_src, out_dst, grp, half, sfx in groups:
                    xg = gxp.tile([P, grp], BF16, tag="gx" + sfx)
                    nc.sync.dma_start(out=xg[:], in_=xg_src)
                    ea = eap.tile([P, half], BF16, tag="ea" + sfx)
                    nc.sync.dma_start(out=ea[:], in_=ea_src)
                    out_t = outp.tile([P, half], BF16, tag="out" + sfx)
                    for p in range(half // SBW):
                        sl = slice(SBW * p, SBW * (p + 1))
                        ps1 = ps1p.tile([P, SBW], F32, tag="p1")
                        nc.tensor.matmul(
                            ps1[0:D], lhsT=w1ab[:], rhs=xg[:, sl],
                            start=True, stop=False,
                        )
                        sl_o = slice(half + SBW * p, half + SBW * (p + 1))
                        nc.tensor.matmul(
                            ps1[D:P], lhsT=w1ab[:], rhs=xg[:, sl_o],
                            start=True, stop=False, skip_group_check=True,
                        )
                        nc.tensor.matmul(
                            ps1[:], lhsT=w1c2[:], rhs=ea[:, sl],
                            start=False, stop=True, skip_group_check=True,
                        )
                        h1 = h1p.tile([P, SBW], BF16, tag="h1")
                        nc.scalar.activation(
                            h1[:], ps1[:], mybir.ActivationFunctionType.Relu,
                            bias=b1d[:], scale=1.0,
                        )
                        if pend is not None:
                            l2_flush(*pend)
                        store = out_dst if p == half // SBW - 1 else None
                        pend = (h1, out_t, sl, store)
                if pend is not None:
                    l2_flush(*pend)
                    pend = None

    nc.compile()
    return nc


def make_in_maps(x, edge_attr, W1, b1, W2, b2, edge_index, n_groups=G,
                 e_shard=E_SHARD):
    """Host-side shard/layout prep. Returns per-core input dicts."""
    e_pad = n_groups * GROUP + GROUP_L
    row = np.asarray(edge_index[0], dtype=np.int64)
    col = np.asarray(edge_index[1], dtype=np.int64)
    x16 = np.asarray(x, dtype=np.float32).astype(bfloat16)
    ea16 = np.asarray(edge_attr, dtype=np.float32).astype(bfloat16)
    W1 = np.asarray(W1, dtype=np.float32)
    w1ab = np.ascontiguousarray(W1[:P].astype(bfloat16))

    def blockdiag(w):
        bd = np.zeros((P, P), bfloat16)
        bd[:D, :D] = w
        bd[D:, D:] = w
        return bd

    w1c2 = blockdiag(W1[P:].astype(bfloat16))
    w22 = blockdiag(np.asarray(W2, dtype=np.float32).astype(bfloat16))
    b1d = np.ascontiguousarray(
        np.tile(np.asarray(b1, dtype=np.float32).reshape(D, 1), (2, 1))
    )
    b2d = np.ascontiguousarray(
        np.tile(np.asarray(b2, dtype=np.float32).reshape(D, 1), (2, 1))
    )
    xT16 = np.ascontiguousarray(x16.T)  # [64, N] for fast column gathers

    def half_stack(ea_s, ngr, half):
        """[E', D] -> [ngr, 128, half]: per group, the two half-group edge
        runs stacked on the partition axis, feature-major."""
        return np.ascontiguousarray(
            ea_s.T.reshape(D, ngr, 2, half)
            .transpose(1, 2, 0, 3)
            .reshape(ngr, P, half)
        )

    e_full = n_groups * GROUP
    in_maps = []
    for c in range(N_CORES):
        sl = slice(c * e_shard, (c + 1) * e_shard)
        row_s = np.zeros(e_pad, np.int64)
        row_s[:e_shard] = row[sl]
        col_s = np.zeros(e_pad, np.int64)
        col_s[:e_shard] = col[sl]
        ea_s = np.zeros((e_pad, D), bfloat16)
        ea_s[:e_shard] = ea16[sl]
        ea2 = half_stack(ea_s[:e_full], n_groups, HALF)
        eal = half_stack(ea_s[e_full:], 1, HALF_L)[0]
        # [G, 128, GROUP]: rows 0-63 = x[row].T, rows 64-127 = x[col].T.
        xg = np.empty((n_groups, P, GROUP), bfloat16)
        rs = row_s[:e_full].reshape(n_groups, GROUP)
        cs = col_s[:e_full].reshape(n_groups, GROUP)
        for g in range(n_groups):
            xg[g, :D] = xT16[:, rs[g]]
            xg[g, D:] = xT16[:, cs[g]]
        xgl = np.empty((P, GROUP_L), bfloat16)
        xgl[:D] = xT16[:, row_s[e_full:]]
        xgl[D:] = xT16[:, col_s[e_full:]]
        in_maps.append({
            "xg": xg,
            "xgl": xgl,
            "ea2": ea2,
            "eal": eal,
            "w1ab": w1ab,
            "w1c2": w1c2,
            "w22": w22,
            "b1d": b1d,
            "b2d": b2d,
        })
    return in_maps


def assemble_output(results, n_groups=G, e_shard=E_SHARD):
    """Invert the feature-major half-stacked layout, concatenate shards."""

    def unstack(o, ngr, half):
        return (
            o.reshape(ngr, 2, D, half // SBW, SBW)
            .transpose(0, 1, 3, 4, 2)
            .reshape(ngr * 2 * half, D)
        )

    outs = []
    for c in range(N_CORES):
        o = unstack(results[c]["out"], n_groups, HALF)
        ol = unstack(results[c]["outl"][None], 1, HALF_L)
        outs.append(np.concatenate([o, ol], axis=0)[:e_shard].astype(np.float32))
    return np.ascontiguousarray(np.concatenate(outs, axis=0))


_NC = None
last_results = None


def kernel(x, edge_attr, W1, b1, W2, b2, edge_index, edge_type):
    global _NC, last_results
    if _NC is None:
        _NC = build_program()
    in_maps = make_in_maps(x, edge_attr, W1, b1, W2, b2, edge_index)
    res = bass_utils.run_bass_kernel_spmd(
        _NC, in_maps, core_ids=list(range(N_CORES))
    )
    last_results = res
    return assemble_output(res.results)
